# revision 30
# baseline (speedup 1.0000x reference)
"""Trainium2 Bass kernel for nn_CandidateAware_ClickedNewsAttention.

Pure data-parallel across 8 NeuronCores: batch B=256 is sharded 32/core,
weights replicated. Inside each core:

  phase 0: weights cast->bf16 in DRAM scratch, DMA-transposed into SBUF;
           cand_topics / clicked_topics cast->bf16 DRAM scratch, transposed.
  phase 1a: QfT = Wq @ cand^T (PE, bf16), query norms -> qw softmax (batched).
  phase 1b: per batch-el: KfT, scores per head (PE quadrant-packed),
            exp (ACT, fused row-sum), weighted head/candidate aggregation
            via PE matvec -> agg; then batched softmax -> attn_weights_agg.
  phase 2: per 128-token tile: clicked_news loaded fp32, transposed on PE,
           H = CN @ Wg^T (PE), gate = sigmoid(aw*H + bg) (ACT, fused scale),
           combine + LayerNorm (DVE/ACT/GPSIMD split), DMA out.

All matmul operands bf16 (fp32 accumulate in PSUM); elementwise fp32.
"""

import math
import os
import sys

import numpy as np

SKIP_ATTN = os.environ.get("K_SKIP_ATTN") == "1"
SKIP_TAIL = os.environ.get("K_SKIP_TAIL") == "1"

sys.path.insert(0, "/opt/trn_rl_repo")

import concourse.bass as bass  # noqa: E402
import concourse.mybir as mybir  # noqa: E402
import concourse.tile as tile  # noqa: E402
from concourse.masks import make_identity  # noqa: E402

F32 = mybir.dt.float32
BF16 = mybir.dt.bfloat16
AF = mybir.ActivationFunctionType
ALU = mybir.AluOpType

NCORES = 8
B, H, N, D, Dt = 256, 200, 64, 640, 256
NH, HD = 10, 64
LN_EPS = 1e-5


def build_nc(BL, H, N, D, Dt, has_bq, has_bk, has_bg, has_gamma, has_beta):
    """Build the per-core Bass program for a local batch of BL elements."""
    assert D % 128 == 0 and Dt % 128 == 0 and N % 32 == 0
    nd = D // 128          # d-tiles (5)
    nkt = Dt // 128        # contraction tiles for projections (2)
    BLH = BL * H           # clicked tokens per core (6400)
    BLN = BL * N           # candidate tokens per core (2048)
    assert BLH % 128 == 0
    TT = BLH // 128        # token tiles for phase 2 (50)
    P32 = 32               # partition padding for the small batched softmaxes
    assert BL <= P32 or BL % 32 == 0
    PB = max(P32, BL)      # partitions used for [b, ...] layouts
    scale = 1.0 / math.sqrt(D)
    G1 = min(8, BL)        # batch group for attention phase
    assert BL % G1 == 0
    G2 = 10 if TT % 10 == 0 else (5 if TT % 5 == 0 else 1)  # stats group

    nc = bass.Bass(target_bir_lowering=True)

    cn = nc.declare_dram_parameter("clicked_news", [BL, H, D], F32, False)
    ct = nc.declare_dram_parameter("clicked_topics", [BL, H, Dt], F32, False)
    cand = nc.declare_dram_parameter("cand_topics", [BL, N, Dt], F32, False)
    wq = nc.declare_dram_parameter("Wq", [D, Dt], F32, False)
    wk = nc.declare_dram_parameter("Wk", [D, Dt], F32, False)
    wg = nc.declare_dram_parameter("Wg", [D, D], F32, False)
    bq = nc.declare_dram_parameter("bq", [D], F32, False) if has_bq else None
    bk = nc.declare_dram_parameter("bk", [D], F32, False) if has_bk else None
    bg = nc.declare_dram_parameter("bg", [D], F32, False) if has_bg else None
    gam = nc.declare_dram_parameter("ln_gamma", [D], F32, False) if has_gamma else None
    bet = nc.declare_dram_parameter("ln_beta", [D], F32, False) if has_beta else None
    out = nc.declare_dram_parameter("out", [BL, H, D], F32, True)
    aw_out = nc.declare_dram_parameter("attn_weights_agg", [BL, H], F32, True)

    # DRAM scratch (bf16 bounce buffers for DMA-transpose)
    scr_wq = nc.dram_tensor("scr_wq", [D, Dt], BF16)
    scr_wk = nc.dram_tensor("scr_wk", [D, Dt], BF16)
    scr_wg = nc.dram_tensor("scr_wg", [D, D], BF16)
    scr_ct = nc.dram_tensor("scr_ct", [BLH, Dt], BF16)
    scr_cand = nc.dram_tensor("scr_cand", [BLN, Dt], BF16)
    scr_qn = nc.dram_tensor("scr_qn", [BLN], F32)
    scr_agg = nc.dram_tensor("scr_agg", [BLH], F32)
    scr_raw = nc.dram_tensor("scr_raw", [BLH], F32) if has_bg else None

    cn_flat = cn[:].flatten_outer_dims()      # [BLH, D]
    ct_flat = ct[:].flatten_outer_dims()      # [BLH, Dt]
    cand_flat = cand[:].flatten_outer_dims()  # [BLN, Dt]
    out_flat = out[:].flatten_outer_dims()    # [BLH, D]

    def qchunks(total, step=512):
        o = 0
        while o < total:
            w = min(step, total - o)
            yield o, w
            o += w

    with tile.TileContext(nc, pool_alloc_mode="queue") as tc:
        with tc.tile_pool(name="consts", bufs=1) as consts:
            # ---- phase 0: casts + transposed weights/activations ----
            nc.gpsimd.dma_start(out=scr_wq[:], in_=wq[:])
            nc.gpsimd.dma_start(out=scr_wk[:], in_=wk[:])
            nc.gpsimd.dma_start(out=scr_wg[:], in_=wg[:])
            nc.gpsimd.dma_start(out=scr_ct[:], in_=ct_flat)
            nc.gpsimd.dma_start(out=scr_cand[:], in_=cand_flat)

            # NOTE: every DMA instruction can carry at most ONE semaphore
            # wait (64B ISA struct; walrus cannot split waits for DMA the
            # way it can for engine instructions). Transposes are grouped
            # per source so each carries exactly its source-cast wait.
            wqT = consts.tile([128, nkt, D], BF16)
            wkT = consts.tile([128, nkt, D], BF16)
            for j in range(nkt):
                nc.sync.dma_start_transpose(
                    out=wqT[:, j, :], in_=scr_wq[:, j * 128:(j + 1) * 128])
            for j in range(nkt):
                nc.sync.dma_start_transpose(
                    out=wkT[:, j, :], in_=scr_wk[:, j * 128:(j + 1) * 128])
            wgT = consts.tile([128, nd, D], BF16)
            for j in range(nd):
                nc.sync.dma_start_transpose(
                    out=wgT[:, j, :], in_=scr_wg[:, j * 128:(j + 1) * 128])
            ctT = consts.tile([128, nkt, BLH], BF16)
            candT = consts.tile([128, nkt, BLN], BF16)
            for j in range(nkt):
                nc.sync.dma_start_transpose(
                    out=ctT[:, j, :], in_=scr_ct[:, j * 128:(j + 1) * 128])
            for j in range(nkt):
                nc.sync.dma_start_transpose(
                    out=candT[:, j, :], in_=scr_cand[:, j * 128:(j + 1) * 128])

            bq_col = bk_col = None
            if has_bq:
                bq_col = consts.tile([128, nd], F32)
                nc.sync.dma_start(
                    out=bq_col, in_=bq[:].rearrange("(t p) -> p t", p=128))
            if has_bk:
                bk_col = consts.tile([128, nd], F32)
                nc.sync.dma_start(
                    out=bk_col, in_=bk[:].rearrange("(t p) -> p t", p=128))
            bg_row = None
            if has_bg:
                bg_row = consts.tile([1, D], BF16)
                nc.gpsimd.dma_start(
                    out=bg_row[0:1, :], in_=bg[:].rearrange("(o d) -> o d", o=1))
            gamB = betB = None
            if has_gamma:
                gamB = consts.tile([128, D], F32)
                nc.sync.dma_start(
                    out=gamB,
                    in_=bass.AP(tensor=gam.tensor if hasattr(gam, "tensor") else gam[:].tensor,
                                offset=0, ap=[[0, 128], [1, D]]))
            if has_beta:
                betB = consts.tile([128, D], F32)
                nc.sync.dma_start(
                    out=betB,
                    in_=bass.AP(tensor=bet.tensor if hasattr(bet, "tensor") else bet[:].tensor,
                                offset=0, ap=[[0, 128], [1, D]]))

            ident = consts.tile([128, 128], F32)
            make_identity(nc, ident)
            ones_col = consts.tile([128, 1], BF16)
            nc.vector.memset(ones_col, 1.0)
            eps_col = consts.tile([128, 1], F32)
            nc.vector.memset(eps_col, LN_EPS)

            qfT = consts.tile([128, nd, BLN], BF16)
            qnorm_row = consts.tile([1, BLN], F32)
            qw_st = consts.tile([128, PB], BF16)
            agg_row = consts.tile([1, BLH], F32)
            aw_all = consts.tile([PB, H], F32)

            # ---- phase 1a: Q projection + query-norm softmax ----
            with tc.tile_pool(name="p1a", bufs=1) as p1a, \
                 tc.tile_pool(name="ps1a", bufs=2, space="PSUM") as ps1a, \
                 tc.tile_pool(name="ps1n", bufs=2, space="PSUM") as ps1n:
                sq = p1a.tile([128, nd, BLN], BF16)
                for dtile in range(nd):
                    for off, w in qchunks(BLN):
                        ps = ps1a.tile([128, 512], F32, tag="qproj")
                        for kt in range(nkt):
                            nc.tensor.matmul(
                                ps[:, 0:w],
                                lhsT=wqT[:, kt, dtile * 128:(dtile + 1) * 128],
                                rhs=candT[:, kt, off:off + w],
                                start=(kt == 0), stop=(kt == nkt - 1))
                        if has_bq:
                            nc.vector.tensor_scalar_add(
                                out=qfT[:, dtile, off:off + w], in0=ps[:, 0:w],
                                scalar1=bq_col[:, dtile:dtile + 1])
                            nc.scalar.activation(
                                out=sq[:, dtile, off:off + w], in_=ps[:, 0:w],
                                func=AF.Square, bias=bq_col[:, dtile:dtile + 1])
                        else:
                            nc.vector.tensor_copy(
                                out=qfT[:, dtile, off:off + w], in_=ps[:, 0:w])
                            nc.scalar.activation(
                                out=sq[:, dtile, off:off + w], in_=ps[:, 0:w],
                                func=AF.Square)
                for off, w in qchunks(BLN):
                    psn = ps1n.tile([1, 512], F32, tag="qnorm")
                    for dtile in range(nd):
                        nc.tensor.matmul(
                            psn[0:1, 0:w], lhsT=ones_col,
                            rhs=sq[:, dtile, off:off + w],
                            start=(dtile == 0), stop=(dtile == nd - 1))
                    nc.scalar.activation(
                        out=qnorm_row[0:1, off:off + w], in_=psn[0:1, 0:w],
                        func=AF.Sqrt)

            # batched softmax over candidates (per batch-el)
            # NB: single-partition DMA rows larger than ~8KB fail NEFF
            # load on this runtime -- chunk all [1, N] row transfers.
            ROWCHUNK = 1024
            for off in range(0, BLN, ROWCHUNK):
                w = min(ROWCHUNK, BLN - off)
                nc.sync.dma_start(
                    out=scr_qn[off:off + w].rearrange("(o t) -> o t", o=1),
                    in_=qnorm_row[0:1, off:off + w])
            with tc.tile_pool(name="qwp", bufs=1) as qwp:
                qn_bn = qwp.tile([PB, N], F32)
                nc.vector.memset(qn_bn, 0.0)
                nc.sync.dma_start(
                    out=qn_bn[0:BL, :], in_=scr_qn[:].rearrange("(b n) -> b n", n=N))
                qe = qwp.tile([PB, N], F32)
                nc.vector.memset(qe, 0.0)
                qsum = qwp.tile([PB, 1], F32)
                nc.scalar.activation(
                    out=qe[0:BL, :], in_=qn_bn[0:BL, :], func=AF.Exp,
                    accum_out=qsum[0:BL, :])
                qrec = qwp.tile([PB, 1], F32)
                nc.vector.reciprocal(out=qrec[0:BL, :], in_=qsum[0:BL, :])
                qw_bn = qwp.tile([PB, N], F32)
                nc.vector.memset(qw_bn, 0.0)
                nc.vector.tensor_scalar_mul(
                    out=qw_bn[0:BL, :], in0=qe[0:BL, :], scalar1=qrec[0:BL, :])
                qwT = qwp.tile([N, PB], F32)
                for jb in range(N // 32):
                    for ib in range(PB // 32):
                        nc.vector.transpose(
                            out=qwT[32 * jb:32 * jb + 32, 32 * ib:32 * ib + 32],
                            in_=qw_bn[32 * ib:32 * ib + 32, 32 * jb:32 * jb + 32])
                for hl in range(2):
                    nc.vector.tensor_copy(
                        out=qw_st[64 * hl:64 * hl + N, :], in_=qwT[0:N, :])

            # ---- phase 1b: K projection, scores, attention aggregation ----
            if SKIP_ATTN:
                nc.vector.memset(agg_row, 0.0)
            with tc.tile_pool(name="kf", bufs=3) as kfp, \
                 tc.tile_pool(name="expp", bufs=G1 + 1) as expp, \
                 tc.tile_pool(name="grp", bufs=2) as grp, \
                 tc.tile_pool(name="psk", bufs=2, space="PSUM") as psk, \
                 tc.tile_pool(name="pss", bufs=2, space="PSUM") as pss, \
                 tc.tile_pool(name="psa", bufs=2, space="PSUM") as psa:
                for b0 in range(0, 0 if SKIP_ATTN else BL, G1):
                    rs = grp.tile([128, G1 * nd], F32, tag="rs")
                    exp_tiles = []
                    for bi in range(G1):
                        b = b0 + bi
                        kfT_b = kfp.tile([128, nd, H], BF16, tag="kfT")
                        exp_b = expp.tile([128, nd, H], BF16, tag="exp")
                        exp_tiles.append(exp_b)
                        for dtile in range(nd):
                            psb = psk.tile([128, H], F32, tag="kproj")
                            for kt in range(nkt):
                                nc.tensor.matmul(
                                    psb,
                                    lhsT=wkT[:, kt, dtile * 128:(dtile + 1) * 128],
                                    rhs=ctT[:, kt, b * H:(b + 1) * H],
                                    start=(kt == 0), stop=(kt == nkt - 1))
                            if has_bk:
                                nc.vector.tensor_scalar_add(
                                    out=kfT_b[:, dtile, :], in0=psb,
                                    scalar1=bk_col[:, dtile:dtile + 1])
                            else:
                                nc.vector.tensor_copy(
                                    out=kfT_b[:, dtile, :], in_=psb)
                        for dtile in range(nd):
                            ps_s = pss.tile([128, H], F32, tag="scores")
                            for hl in range(2):
                                h0 = 64 * hl
                                nc.tensor.matmul(
                                    ps_s[h0:h0 + 64, :],
                                    lhsT=qfT[h0:h0 + 64, dtile, b * N:(b + 1) * N],
                                    rhs=kfT_b[h0:h0 + 64, dtile, :],
                                    start=True, stop=True,
                                    tile_position=(h0, h0))
                            nc.scalar.activation(
                                out=exp_b[:, dtile, :], in_=ps_s, func=AF.Exp,
                                scale=scale,
                                accum_out=rs[:, bi * nd + dtile:bi * nd + dtile + 1])
                    rr = grp.tile([128, G1 * nd], F32, tag="rr")
                    nc.vector.reciprocal(out=rr, in_=rs)
                    w_all = grp.tile([128, G1, nd], BF16, tag="w")
                    qsl = qw_st[:, b0:b0 + G1]
                    qw_b = bass.AP(tensor=qsl.tensor, offset=qsl.offset,
                                   ap=list(qsl.ap) + [[0, nd]])
                    nc.vector.tensor_tensor(
                        out=w_all,
                        in0=rr[:].rearrange("p (b t) -> p b t", t=nd),
                        in1=qw_b, op=ALU.mult)
                    for bi in range(G1):
                        b = b0 + bi
                        agg_ps = psa.tile([1, H], F32, tag="agg")
                        for dtile in range(nd):
                            nc.tensor.matmul(
                                agg_ps, lhsT=w_all[:, bi, dtile:dtile + 1],
                                rhs=exp_tiles[bi][:, dtile, :],
                                start=(dtile == 0), stop=(dtile == nd - 1))
                        nc.scalar.activation(
                            out=agg_row[0:1, b * H:(b + 1) * H], in_=agg_ps,
                            func=AF.Copy)

            # ---- attn_weights_agg = softmax(agg) over clicked dim ----
            for off in range(0, BLH, ROWCHUNK):
                w = min(ROWCHUNK, BLH - off)
                nc.sync.dma_start(
                    out=scr_agg[off:off + w].rearrange("(o t) -> o t", o=1),
                    in_=agg_row[0:1, off:off + w])
            with tc.tile_pool(name="awp", bufs=1) as awp:
                agg_all = awp.tile([PB, H], F32)
                nc.gpsimd.memset(agg_all, 0.0)
                nc.sync.dma_start(
                    out=agg_all[0:BL, :],
                    in_=scr_agg[:].rearrange("(b h) -> b h", h=H))
                ae = awp.tile([PB, H], F32)
                asum = awp.tile([PB, 1], F32)
                nc.scalar.activation(
                    out=ae[0:BL, :], in_=agg_all[0:BL, :], func=AF.Exp,
                    accum_out=asum[0:BL, :])
                arec = awp.tile([PB, 1], F32)
                nc.vector.reciprocal(out=arec[0:BL, :], in_=asum[0:BL, :])
                nc.vector.tensor_scalar_mul(
                    out=aw_all[0:BL, :], in0=ae[0:BL, :], scalar1=arec[0:BL, :])
                nc.sync.dma_start(out=aw_out[:], in_=aw_all[0:BL, :])
                if has_bg:
                    raw = awp.tile([PB, H], F32)
                    nc.vector.reciprocal(out=raw[0:BL, :], in_=aw_all[0:BL, :])
                    nc.sync.dma_start(out=scr_raw[:], in_=raw[0:BL, :])

            # aw in token order, as per-partition columns [128, TT]
            awcol = consts.tile([128, TT], F32)
            nc.sync.dma_start(
                out=awcol, in_=aw_out[:].rearrange("b h -> (b h)").rearrange(
                    "(t p) -> p t", p=128))
            awm1 = consts.tile([128, TT], F32)
            nc.vector.tensor_scalar_add(out=awm1, in0=awcol, scalar1=-1.0)
            raw_row = None
            if has_bg:
                raw_row = consts.tile([1, BLH], BF16)
                for off in range(0, BLH, ROWCHUNK):
                    w = min(ROWCHUNK, BLH - off)
                    nc.gpsimd.dma_start(
                        out=raw_row[0:1, off:off + w],
                        in_=scr_raw[off:off + w].rearrange("(o t) -> o t", o=1))

            xsum = consts.tile([128, TT], F32)
            x2sum = consts.tile([128, TT], F32)
            mean_all = consts.tile([128, TT], F32)
            rstd_all = consts.tile([128, TT], F32)

            # ---- phase 2: gate GEMM + combine + LayerNorm, token-tiled ----
            with tc.tile_pool(name="cnp", bufs=3) as cnp, \
                 tc.tile_pool(name="cntp", bufs=2) as cntp, \
                 tc.tile_pool(name="gatep", bufs=2) as gatep, \
                 tc.tile_pool(name="prep", bufs=G2 + 2) as prep, \
                 tc.tile_pool(name="sqp", bufs=2) as sqp, \
                 tc.tile_pool(name="outp", bufs=3) as outp, \
                 tc.tile_pool(name="stp", bufs=2) as stp, \
                 tc.tile_pool(name="pst", bufs=2, space="PSUM") as pst, \
                 tc.tile_pool(name="psh", bufs=2, space="PSUM") as psh:
                if SKIP_TAIL:
                    for g in range(TT):
                        out_g = outp.tile([128, D], F32, tag="out")
                        nc.gpsimd.memset(out_g, 0.0)
                        nc.sync.dma_start(
                            out=out_flat[g * 128:(g + 1) * 128, :], in_=out_g)
                for g0 in range(0, 0 if SKIP_TAIL else TT, G2):
                    pre_tiles = []
                    for g in range(g0, g0 + G2):
                        cn_g = cnp.tile([128, D], F32, tag="cn")
                        # engine-op pre-writer: absorbs the slot-recycle WAR
                        # waits (PE transposes + DVE stt read this slot) so
                        # the load DMA itself needs only one wait
                        nc.gpsimd.memset(cn_g, 0.0)
                        # scalar-engine HWDGE ring: keeps each of the two
                        # dynamic HWDGE rings under its entry cap
                        nc.scalar.dma_start(
                            out=cn_g, in_=cn_flat[g * 128:(g + 1) * 128, :])
                        tp = pst.tile([128, D], F32, tag="tp")
                        for j in range(nd):
                            nc.tensor.transpose(
                                out=tp[:, j * 128:(j + 1) * 128],
                                in_=cn_g[:, j * 128:(j + 1) * 128],
                                identity=ident)
                        cnT_g = cntp.tile([128, nd, 128], BF16, tag="cnT")
                        nc.vector.tensor_copy(
                            out=cnT_g[:].rearrange("p a b -> p (a b)"), in_=tp)
                        h_ps = psh.tile([128, D], F32, tag="h")
                        for e0, ew in qchunks(D):
                            if has_bg:
                                nc.tensor.matmul(
                                    h_ps[:, e0:e0 + ew],
                                    lhsT=raw_row[0:1, g * 128:(g + 1) * 128],
                                    rhs=bg_row[0:1, e0:e0 + ew],
                                    start=True, stop=False)
                            for j in range(nd):
                                nc.tensor.matmul(
                                    h_ps[:, e0:e0 + ew], lhsT=cnT_g[:, j, :],
                                    rhs=wgT[:, j, e0:e0 + ew],
                                    start=(j == 0 and not has_bg),
                                    stop=(j == nd - 1))
                        gate_g = gatep.tile([128, D], F32, tag="gate")
                        nc.scalar.activation(
                            out=gate_g, in_=h_ps, func=AF.Sigmoid,
                            scale=awcol[:, g:g + 1])
                        t_g = gatep.tile([128, D], F32, tag="t")
                        nc.gpsimd.tensor_scalar(
                            out=t_g, in0=gate_g,
                            scalar1=awm1[:, g:g + 1], scalar2=1.0,
                            op0=ALU.mult, op1=ALU.add)
                        pre_g = prep.tile([128, D], F32, tag="pre")
                        pre_tiles.append(pre_g)
                        nc.vector.scalar_tensor_tensor(
                            out=pre_g, in0=t_g, scalar=1.0, in1=cn_g,
                            op0=ALU.mult, op1=ALU.mult,
                            accum_out=xsum[:, g:g + 1])
                        sq_g = sqp.tile([128, D], BF16, tag="sq")
                        nc.scalar.activation(
                            out=sq_g, in_=pre_g, func=AF.Square,
                            accum_out=x2sum[:, g:g + 1])
                    # batched LN stats for the group
                    gsl = slice(g0, g0 + G2)
                    nc.vector.tensor_scalar_mul(
                        out=mean_all[:, gsl], in0=xsum[:, gsl], scalar1=1.0 / D)
                    m2 = stp.tile([128, G2], F32, tag="m2")
                    nc.vector.tensor_tensor(
                        out=m2, in0=mean_all[:, gsl], in1=mean_all[:, gsl],
                        op=ALU.mult)
                    var = stp.tile([128, G2], F32, tag="var")
                    nc.vector.scalar_tensor_tensor(
                        out=var, in0=x2sum[:, gsl], scalar=1.0 / D, in1=m2,
                        op0=ALU.mult, op1=ALU.subtract)
                    std = stp.tile([128, G2], F32, tag="std")
                    nc.scalar.activation(
                        out=std, in_=var, func=AF.Sqrt, bias=eps_col)
                    nc.vector.reciprocal(out=rstd_all[:, gsl], in_=std)
                    for gi, g in enumerate(range(g0, g0 + G2)):
                        out_g = outp.tile([128, D], F32, tag="out")
                        nc.gpsimd.tensor_scalar(
                            out=out_g, in0=pre_tiles[gi],
                            scalar1=mean_all[:, g:g + 1],
                            scalar2=rstd_all[:, g:g + 1],
                            op0=ALU.subtract, op1=ALU.mult)
                        if has_gamma:
                            nc.gpsimd.tensor_tensor(
                                out=out_g, in0=out_g, in1=gamB, op=ALU.mult)
                        if has_beta:
                            nc.gpsimd.tensor_tensor(
                                out=out_g, in0=out_g, in1=betB, op=ALU.add)
                        nc.sync.dma_start(
                            out=out_flat[g * 128:(g + 1) * 128, :], in_=out_g)
    legalize_waits(nc)
    return nc


def legalize_waits(nc, max_inline=1):
    """Split multi-wait instructions for this toolchain's walrus.

    Every 64B TPB instruction has ONE wait slot; this walrus version
    refuses instructions with more ("Too many sync wait commands").
    Tile's wait pass emits several. Since all waits (including HWDGE
    DMA waits on TRN2) execute on the issuing engine's sequencer,
    hoisting excess waits into standalone InstEventSemaphore
    instructions placed immediately before, on the same engine, is
    semantics-preserving.
    """
    n_split = 0
    for f in nc.m.functions:
        for bb in f.blocks:
            new = []
            changed = False
            for ins in bb.instructions:
                si = ins.sync_info
                if si is not None and len(si.on_wait) > max_inline:
                    waits = list(si.on_wait)
                    keep = waits[-max_inline:] if max_inline else []
                    hoist = waits[:len(waits) - max_inline]
                    for k, w in enumerate(hoist):
                        assert w.wait_reg is None, f"reg wait on {ins.name}"
                        ev = mybir.InstEventSemaphore(
                            name=f"{ins.name}-hw{k}", engine=ins.engine,
                            ins=[], outs=[],
                            sync_info=mybir.SyncInfo(on_wait=[w], on_update=[]))
                        nc.register_instruction(ev, overwrite=True)
                        new.append(ev)
                        n_split += 1
                    ins.sync_info = mybir.SyncInfo(
                        on_wait=keep, on_update=list(si.on_update))
                    changed = True
                new.append(ins)
            if changed:
                bb.instructions = new
    return n_split


_cache = {}


def _get_nc(key):
    if key not in _cache:
        _cache[key] = build_nc(*key)
    return _cache[key]


def kernel(**inputs):
    inp = {k: np.asarray(v) for k, v in inputs.items()}
    has_bq = bool(np.any(inp["bq"]))
    has_bk = bool(np.any(inp["bk"]))
    has_bg = bool(np.any(inp["bg"]))
    has_gamma = bool(np.any(inp["ln_gamma"] != 1.0))
    has_beta = bool(np.any(inp["ln_beta"]))
    BL = B // NCORES
    nc = _get_nc((BL, H, N, D, Dt, has_bq, has_bk, has_bg, has_gamma, has_beta))

    in_maps = []
    for i in range(NCORES):
        sl = slice(i * BL, (i + 1) * BL)
        m = {
            "clicked_news": np.ascontiguousarray(inp["clicked_news"][sl], np.float32),
            "clicked_topics": np.ascontiguousarray(inp["clicked_topics"][sl], np.float32),
            "cand_topics": np.ascontiguousarray(inp["cand_topics"][sl], np.float32),
            "Wq": np.ascontiguousarray(inp["Wq"], np.float32),
            "Wk": np.ascontiguousarray(inp["Wk"], np.float32),
            "Wg": np.ascontiguousarray(inp["Wg"], np.float32),
        }
        if has_bq:
            m["bq"] = np.ascontiguousarray(inp["bq"], np.float32)
        if has_bk:
            m["bk"] = np.ascontiguousarray(inp["bk"], np.float32)
        if has_bg:
            m["bg"] = np.ascontiguousarray(inp["bg"], np.float32)
        if has_gamma:
            m["ln_gamma"] = np.ascontiguousarray(inp["ln_gamma"], np.float32)
        if has_beta:
            m["ln_beta"] = np.ascontiguousarray(inp["ln_beta"], np.float32)
        in_maps.append(m)

    global _last_in_maps
    _last_in_maps = in_maps
    from concourse.bass_utils import run_bass_kernel_spmd
    res = run_bass_kernel_spmd(nc, in_maps, list(range(NCORES)))
    out = np.concatenate([r["out"] for r in res.results], axis=0)
    aw = np.concatenate([r["attn_weights_agg"] for r in res.results], axis=0)
    return out.astype(np.float32), aw.astype(np.float32)
